# revision 33
# baseline (speedup 1.0000x reference)
"""Trainium2 Bass kernel: ConvolutionalMultiheadAttention.

Reference computation (per batch element b):
    q = conv1d(x, w0) + b0          # [D, Lp]  (VALID, K=3)
    k = conv1d(x, w1) + b1
    v = conv1d(x, w2) + b2
    per head h (Dh=64): out_h = v_h @ softmax(q_h^T k_h / sqrt(D))^T

Sharding: data-parallel over batch B=16 across 8 cores (2 per core).
Weights replicated. No collectives.

Per-core kernel architecture v2 (PE-roofline oriented):
  - conv as matmul: contraction over input channel i (4 chunks of 128),
    12 accumulating matmuls per PSUM tile. q,k in [o_part, t_free]
    layout; v transposed [t_part, o_free].
  - v tile layout [t_part, ktc, h, 128]: cols 0..63 constant 1.0, cols
    64..127 hold v. The AV matmul yields the softmax denominator on
    PSUM partition 0 and the output block on partitions 64..127 for
    free (matmul cost is column-streaming-bound, M-independent).
  - scores PAIRED: heads 2p (partitions 0:64 of oc=p) and 2p+1 (64:128)
    are emitted as alternating matmuls with row-tile positions (0,0)
    and (64,0) (auto-derived from base_partition). The two K=64 MMs
    occupy disjoint row groups of the PE array and run CONCURRENTLY,
    halving score stream time. exp via ACT with the 1/sqrt(512) scale
    folded in, fp16 out.
  - unified fine-grained scheduler: score chunks (4 MMs + 2 exps each)
    are emitted every ~FILL_NS of conv/AV filler work so the 2-buf
    score-PSUM ring never stalls the PE behind ACT (2x1.11us exp per
    chunk), the ACT engine starts its 142us of exp work at ~20us, and
    AV units drain the pt ring one pair behind the score stream.
  - HAM pre-warm: WARMUP dummy matmuls on a memset tile keep the PE
    busy from the end of the engine preamble (~7us) through the DMA
    load window so the conv stream starts at 2.4GHz (K=8/8) instead of
    paying the cold 1.2GHz clock until ~30us.
  - input DMA: sync issues the first-need-critical stream (wq0 kk
    slices, x0 a-halves, wk0); scalar issues x0 b-halves + remaining
    weights in parallel (each dma_start costs ~600ns of serial
    descriptor-write time on its issuing engine); gpsimd issues biases
    and the bulk wv/x1 loads gated behind the first conv output.
"""

import numpy as np

import concourse.bass as bass
import concourse.bacc as bacc
import concourse.mybir as mybir
import concourse.tile as tile
from concourse.bass_utils import run_bass_kernel_spmd

B, D, L, KW, H = 16, 512, 1024, 3, 8
LP = L - KW + 1          # 1022
DH = D // H              # 64
NCORES = 8
BLOC = B // NCORES       # 2
NIC = D // 128           # 4 input-channel chunks
SCALE = 1.0 / float(np.sqrt(D))
import os
MM_DTYPE_NAME = os.environ.get('MM_DTYPE', 'bf16')

F32 = mybir.dt.float32
F32R = mybir.dt.float32r
F16 = mybir.dt.float16
BF16 = mybir.dt.bfloat16
MMDT = {"f32r": F32R, "bf16": BF16, "f32": F32}[MM_DTYPE_NAME]
import ml_dtypes
MMDT_NP = {"f32r": np.float32, "bf16": ml_dtypes.bfloat16, "f32": np.float32}[MM_DTYPE_NAME]

# time chunking
TQ = [(0, 512), (512, LP - 512)]                       # qt chunks (512, 510)
TKC = [(i * 128, min(128, LP - i * 128)) for i in range(8)]  # kt chunks (...126)

PT_BUFS = int(os.environ.get('PT_BUFS', '32'))
WARMUP = int(os.environ.get('WARMUP', '24'))
FILL_NS = int(os.environ.get('FILL_NS', '2550'))
SCALAR_DMA = int(os.environ.get('SCALAR_DMA', '1'))

# PE-time cost model for pacing (ns): ~244ns per N=512 matmul slot
MM_NS = 244


def _emit(tc, xs, wq, wk, wv, bq, bk, bv, out):
    nc = tc.nc
    Exp = mybir.ActivationFunctionType.Exp
    from concourse.alu_op_type import AluOpType
    Add = AluOpType.add
    from contextlib import ExitStack
    ctx = ExitStack()
    wpool = ctx.enter_context(tc.tile_pool(name="w", bufs=1))
    cpool = ctx.enter_context(tc.tile_pool(name="const", bufs=1))
    xpool = ctx.enter_context(tc.tile_pool(name="x", bufs=1))
    qkpool = ctx.enter_context(tc.tile_pool(name="qk", bufs=1))
    vpool = ctx.enter_context(tc.tile_pool(name="v", bufs=2))
    ptpool = ctx.enter_context(tc.tile_pool(name="pt", bufs=PT_BUFS))
    opool = ctx.enter_context(tc.tile_pool(name="o", bufs=6))
    rpool = ctx.enter_context(tc.tile_pool(name="r", bufs=4))
    bpool = ctx.enter_context(tc.tile_pool(name="bc", bufs=4))
    # PSUM (8 banks): pscore 2x[128,1024] (4) + pconv 2x[128,512] (2)
    # + pav 2x[128,512] (2)
    pscore = ctx.enter_context(tc.tile_pool(name="pscore", bufs=2, space="PSUM"))
    pconv = ctx.enter_context(tc.tile_pool(name="pconv", bufs=2, space="PSUM"))
    pav = ctx.enter_context(tc.tile_pool(name="pav", bufs=2, space="PSUM"))

    # ---- loads: two parallel issuers (sync: first-need critical;
    # scalar: second-wave), gpsimd: biases + gated bulk ----
    wq_ic = []
    wk_ic = []
    x_t = [[None] * NIC for _ in range(BLOC)]
    x0r = xs[0].rearrange("(c p) t -> p c t", p=128)
    x1r = xs[1].rearrange("(c p) t -> p c t", p=128)

    for ic in range(NIC):
        wq_ic.append(wpool.tile([128, KW, D], MMDT, tag=f"wq{ic}", name=f"wq{ic}"))
        wk_ic.append(wpool.tile([128, KW, D], MMDT, tag=f"wk{ic}", name=f"wk{ic}"))
        # overlapping halves: xa covers cols 0..516, xb covers 512..1024,
        # so every conv matmul depends on exactly ONE x DMA
        x_t[0][ic] = (
            xpool.tile([128, 516], MMDT, tag=f"x0{ic}a", name=f"x0{ic}a"),
            xpool.tile([128, 512], MMDT, tag=f"x0{ic}b", name=f"x0{ic}b"),
        )
    # three parallel issue queues, each in first-need order:
    #   sync:   wq0 kk-slices, x0 a-halves, wk0, x0 b-halves (~1.8MB)
    #   gpsimd: biases, wq1, wk1 (~0.8MB)
    #   scalar: wq2, wk2, wq3, wk3 (late needs, ~1.6MB)
    for kk in range(KW):
        nc.sync.dma_start(wq_ic[0][:, kk], wq[:, 0, kk])
    nc.sync.dma_start(x_t[0][0][0][:], x0r[:, 0, 0:516])
    nc.sync.dma_start(x_t[0][1][0][:], x0r[:, 1, 0:516])
    nc.sync.dma_start(x_t[0][2][0][:], x0r[:, 2, 0:516])
    if SCALAR_DMA:
        nc.scalar.dma_start(x_t[0][3][0][:], x0r[:, 3, 0:516])
        nc.scalar.dma_start(wk_ic[0][:], wk[:, 0])
    else:
        nc.sync.dma_start(x_t[0][3][0][:], x0r[:, 3, 0:516])
        nc.sync.dma_start(wk_ic[0][:], wk[:, 0])
    bq_sb = cpool.tile([128, NIC], F32, tag="bq")
    nc.gpsimd.dma_start(bq_sb[:], bq[:])
    bk_sb = cpool.tile([128, NIC], F32, tag="bk")
    nc.gpsimd.dma_start(bk_sb[:], bk[:])
    for ic in range(NIC):
        nc.gpsimd.dma_start(x_t[0][ic][1][:], x0r[:, ic, 512:L])
    nc.gpsimd.dma_start(wq_ic[1][:], wq[:, 1])
    nc.gpsimd.dma_start(wk_ic[1][:], wk[:, 1])
    wv_sb = wpool.tile([128, NIC, KW, D], MMDT, tag="wv")
    bv_sb = cpool.tile([128, D], F32, tag="bv")
    for ic in range(NIC):
        x_t[1][ic] = (
            xpool.tile([128, 516], MMDT, tag=f"x1{ic}a", name=f"x1{ic}a"),
            xpool.tile([128, 512], MMDT, tag=f"x1{ic}b", name=f"x1{ic}b"),
        )

    def load_bulk_wv():
        # gate the 1.5MB wv load + second-wave weights on the first conv
        # output: the input stream is aggregate-HBM-bandwidth-bound
        # across all 8 cores, so bytes not needed until ~50us must not
        # compete with the critical wq0/wk0/x0/wq1/wk1 transfers
        gate = cpool.tile([1, 8], MMDT, tag="gate")
        from concourse.alu_op_type import AluOpType as _A
        nc.gpsimd.tensor_tensor(gate[0:1, :], q_oc[0][0:1, 0:8],
                                q_oc[0][0:1, 0:8], op=_A.add)
        nc.gpsimd.dma_start(wv_sb[:], wv[:])
        nc.gpsimd.dma_start(bv_sb[:], bv[:])
        nc.gpsimd.dma_start(wq_ic[3][:], wq[:, 3])
        nc.gpsimd.dma_start(wk_ic[3][:], wk[:, 3])
        # scalar carries oc2 in parallel, gated the same way
        sgate = cpool.tile([1, 8], F16, tag="sgate")
        nc.scalar.activation(sgate[0:1, :], q_oc[0][0:1, 0:8],
                             mybir.ActivationFunctionType.Exp)
        nc.scalar.dma_start(wq_ic[2][:], wq[:, 2])
        nc.scalar.dma_start(wk_ic[2][:], wk[:, 2])

    def load_bulk_x1():
        for ic in range(NIC):
            nc.gpsimd.dma_start(x_t[1][ic][0][:], x1r[:, ic, 0:516])
            nc.gpsimd.dma_start(x_t[1][ic][1][:], x1r[:, ic, 512:L])

    def x_slice(b, ic, g0, gn):
        xa, xb = x_t[b][ic]
        if g0 + gn <= 516:
            return xa[:, g0:g0 + gn]
        assert g0 >= 512
        return xb[:, g0 - 512:g0 - 512 + gn]

    def w_slice(nm, ic, kk, osl):
        if nm == "q":
            return wq_ic[ic][:, kk, osl]
        if nm == "k":
            return wk_ic[ic][:, kk, osl]
        return wv_sb[:, ic, kk, osl]

    # q/k: per-oc tiles [p, t] with o = oc*128+p
    q_oc = [qkpool.tile([128, L], MMDT, tag=f"q{oc}", name=f"q{oc}") for oc in range(NIC)]
    k_oc = [qkpool.tile([128, L], MMDT, tag=f"k{oc}", name=f"k{oc}") for oc in range(NIC)]
    v_tiles = [None, None]

    # ---- HAM pre-warm: dummy matmuls on a memset tile keep the PE
    # busy through the DMA-load window so real matmuls start warm ----
    wu = cpool.tile([128, 512], MMDT, tag="wu")
    nc.vector.memset(wu[:], 0.0)

    def wu_fill(n):
        for _ in range(n):
            pw = pav.tile([128, 512], F32, tag="pa", name="pw")
            nc.tensor.matmul(pw[:, :], wu[:, 0:128], wu[:, :],
                             start=True, stop=True)

    wu_fill(WARMUP)

    # ---- building blocks ----
    bias_flip = [0]

    def conv_qk_half(b, nm, oc, ti):
        t0, tn = TQ[ti]
        dst = (q_oc if nm == "q" else k_oc)[oc]
        bias_sb = bq_sb if nm == "q" else bk_sb
        ps = pconv.tile([128, 512], F32, tag="pc", name="pc")
        mm = 0
        for ic in range(NIC):
            for kk in range(KW):
                nc.tensor.matmul(
                    ps[:, :tn],
                    w_slice(nm, ic, kk, slice(oc * 128, (oc + 1) * 128)),
                    x_slice(b, ic, t0 + kk, tn),
                    start=(mm == 0), stop=(mm == NIC * KW - 1),
                )
                mm += 1
        nc.vector.tensor_tensor(
            dst[:, t0:t0 + tn], ps[:, :tn],
            bias_sb[:, oc:oc + 1].broadcast_to([128, tn]), op=Add,
        )

    def v_alloc(b):
        v_sb = vpool.tile([128, 8, H, 2 * DH], F16, tag="v")
        nc.vector.memset(v_sb[:, :, :, 0:DH], 1.0)
        v_tiles[b] = v_sb

    def conv_v_piece(b, tci):
        t0, tn = TKC[tci]
        ps = pconv.tile([128, 512], F32, tag="pc", name="pc")
        mm = 0
        for ic in range(NIC):
            for kk in range(KW):
                nc.tensor.matmul(
                    ps[:tn, :],
                    x_slice(b, ic, t0 + kk, tn),
                    wv_sb[:, ic, kk, :],
                    start=(mm == 0), stop=(mm == NIC * KW - 1),
                )
                mm += 1
        nc.vector.tensor_tensor(
            v_tiles[b][:tn, tci, :, DH:2 * DH],
            ps[:tn].rearrange("p (h d) -> p h d", h=H),
            bv_sb[:tn].rearrange("p (h d) -> p h d", h=H),
            op=Add,
        )

    # ---- attention ----
    pt_tiles = {}

    def score_chunk(b, p, tci):
        # paired scores for heads (2p, 2p+1): alternating matmuls at
        # row-tile positions (0,0)/(64,0) run concurrently in the PE
        oc = p
        kt0, ktn = TKC[tci]
        kh_e = k_oc[oc][0:DH, :]
        qh_e = q_oc[oc][0:DH, :]
        kh_o = k_oc[oc][DH:2 * DH, :]
        qh_o = q_oc[oc][DH:2 * DH, :]
        pt_e = ptpool.tile([128, L], F16, tag="pt")
        pt_o = ptpool.tile([128, L], F16, tag="pt")
        ss_e = pscore.tile([128, 1024], F32, tag="ps", name="ss")
        ss_o = pscore.tile([128, 1024], F32, tag="ps", name="ss")
        for (qt0, qtn) in TQ:
            nc.tensor.matmul(
                ss_e[:ktn, qt0:qt0 + qtn],
                kh_e[:, kt0:kt0 + ktn],
                qh_e[:, qt0:qt0 + qtn],
                start=True, stop=True,
            )
            nc.tensor.matmul(
                ss_o[:ktn, qt0:qt0 + qtn],
                kh_o[:, kt0:kt0 + ktn],
                qh_o[:, qt0:qt0 + qtn],
                start=True, stop=True,
            )
        nc.scalar.activation(pt_e[:ktn, 0:LP], ss_e[:ktn, 0:LP], Exp,
                             scale=SCALE)
        nc.scalar.activation(pt_o[:ktn, 0:LP], ss_o[:ktn, 0:LP], Exp,
                             scale=SCALE)
        pt_tiles.setdefault((b, 2 * p), []).append(pt_e)
        pt_tiles.setdefault((b, 2 * p + 1), []).append(pt_o)

    def av_epilogue(pa, b, h, ti):
        qt0, qtn = TQ[ti]
        rec = rpool.tile([1, 512], F32, tag="rec")
        nc.vector.reciprocal_approx_fast(rec[:1, :qtn], pa[0:1, :qtn])
        brd = bpool.tile([DH, 512], F32, tag="brd")
        nc.gpsimd.partition_broadcast(brd[:, :qtn], rec[:1, :qtn])
        ot = opool.tile([DH, 512], BF16, tag="ot")
        nc.vector.tensor_mul(ot[:, :qtn], pa[DH:2 * DH, :qtn], brd[:, :qtn])
        nc.sync.dma_start(
            out[b, DH * h:DH * (h + 1), qt0:qt0 + qtn], ot[:, :qtn]
        )

    def av_qt(b, h, ti, pool=None):
        qt0, qtn = TQ[ti]
        tiles = pt_tiles[(b, h)]
        # tag must match the pool's existing ring ("pc" in pconv) or the
        # pool footprint doubles and PSUM overflows
        pa = (pconv.tile([128, 512], F32, tag="pc", name="pc")
              if pool is pconv else
              pav.tile([128, 512], F32, tag="pa", name="pa"))
        for tci, (kt0, ktn) in enumerate(TKC):
            nc.tensor.matmul(
                pa[:2 * DH, :qtn],
                v_tiles[b][:ktn, tci, h, :],
                tiles[tci][:ktn, qt0:qt0 + qtn],
                start=(tci == 0), stop=(tci == len(TKC) - 1),
            )
        av_epilogue(pa, b, h, ti)
        if ti == 1:
            del pt_tiles[(b, h)]

    # ---- planner: filler units with costs, pair-prereq barriers ----
    def qk_unit(b, nm, oc, ti):
        return (MM_NS * 12, lambda: conv_qk_half(b, nm, oc, ti))

    def v_unit(b, tci):
        return (MM_NS * 12, lambda: conv_v_piece(b, tci))

    def misc_unit(fn):
        return (0, fn)

    F = []
    req = {}  # pair -> required F prefix length
    # b0: v pieces early (so pair-0 AVs can flow during pair 1), then
    # remaining q/k convs; wv/x1 bulk-load triggers first
    F += [qk_unit(0, "q", 1, 0), qk_unit(0, "q", 1, 1),
          misc_unit(load_bulk_wv),
          qk_unit(0, "k", 1, 0), qk_unit(0, "k", 1, 1)]
    req[(0, 1)] = len(F)
    F += [misc_unit(load_bulk_x1), misc_unit(lambda: v_alloc(0)),
          v_unit(0, 0), v_unit(0, 1), v_unit(0, 2), v_unit(0, 3),
          v_unit(0, 4), v_unit(0, 5), v_unit(0, 6), v_unit(0, 7)]
    v_done_idx = {0: len(F)}
    F += [qk_unit(0, "q", 2, 0), qk_unit(0, "q", 2, 1),
          qk_unit(0, "k", 2, 0), qk_unit(0, "k", 2, 1)]
    req[(0, 2)] = len(F)
    F += [qk_unit(0, "q", 3, 0), qk_unit(0, "q", 3, 1),
          qk_unit(0, "k", 3, 0), qk_unit(0, "k", 3, 1)]
    req[(0, 3)] = len(F)
    F += [qk_unit(1, "q", 0, 0), qk_unit(1, "q", 0, 1),
          qk_unit(1, "k", 0, 0), qk_unit(1, "k", 0, 1)]
    req[(1, 0)] = len(F)
    F += [misc_unit(lambda: v_alloc(1)),
          v_unit(1, 0), v_unit(1, 1), v_unit(1, 2), v_unit(1, 3),
          v_unit(1, 4), v_unit(1, 5), v_unit(1, 6), v_unit(1, 7)]
    v_done_idx[1] = len(F)
    F += [qk_unit(1, "q", 1, 0), qk_unit(1, "q", 1, 1),
          qk_unit(1, "k", 1, 0), qk_unit(1, "k", 1, 1)]
    req[(1, 1)] = len(F)
    F += [qk_unit(1, "q", 2, 0), qk_unit(1, "q", 2, 1),
          qk_unit(1, "k", 2, 0), qk_unit(1, "k", 2, 1)]
    req[(1, 2)] = len(F)
    F += [qk_unit(1, "q", 3, 0), qk_unit(1, "q", 3, 1),
          qk_unit(1, "k", 3, 0), qk_unit(1, "k", 3, 1)]
    req[(1, 3)] = len(F)

    # prelude: b0 q0/k0 conv (required by pair (0,0)); t0 pieces first
    # (they only need the x a-halves, which arrive before the b-halves).
    # Warmup matmuls interleave so DMA-arrival stalls inside the prelude
    # never idle the PE past the ~3.4us HAM re-throttle window.
    conv_qk_half(0, "q", 0, 0)
    wu_fill(4)
    conv_qk_half(0, "k", 0, 0)
    wu_fill(4)
    conv_qk_half(0, "q", 0, 1)
    wu_fill(4)
    conv_qk_half(0, "k", 0, 1)

    PAIRS = [(0, 0), (0, 1), (0, 2), (0, 3),
             (1, 0), (1, 1), (1, 2), (1, 3)]
    from collections import deque
    avq = deque()          # entries: (pair_idx, b, h, ti)
    pending_av = deque()
    fi = 0
    debt = 0

    def promote_pending():
        # move avs whose batch's v conv is fully emitted into the queue
        while pending_av and fi >= v_done_idx[pending_av[0][1]]:
            avq.append(pending_av.popleft())

    def emit_f():
        nonlocal fi
        cost, fn = F[fi]
        fn()
        fi += 1
        promote_pending()
        return cost

    for pi, (b, p) in enumerate(PAIRS):
        # pt-ring safety: pair pi's chunks reuse the pt bufs of pair
        # pi-2; those readers (pair pi-2's AV matmuls) MUST already be
        # in the PE stream or the ring-writer wait deadlocks. Interleave
        # the forced AVs with the required conv units so consecutive AV
        # epilogues don't serialize on gpsimd-bcast/mult/out-DMA.
        while fi < req.get((b, p), 0) or (avq and avq[0][0] <= pi - 2):
            if fi < req.get((b, p), 0):
                emit_f()
            if avq and avq[0][0] <= pi - 2:
                _, bb, hh, ti = avq.popleft()
                av_qt(bb, hh, ti)
        assert not any(e[0] <= pi - 2 for e in pending_av), (
            f"pt-ring deadlock: pair {pi} needs avs of pair {pi-2} "
            f"but v conv not yet emitted")
        last_pair = pi == len(PAIRS) - 1
        if last_pair:
            # final pair: the even head's AV accumulation groups stay
            # OPEN in the pav banks and receive one matmul per score
            # chunk as its exp lands, instead of waiting for the whole
            # pair. Reserve AVs drain through the (now idle) pconv
            # banks. This fills the ACT-paced terminal chunk-gaps and
            # shortens the post-exp drain.
            pa_last = [pav.tile([128, 512], F32, tag="pa", name=f"paL{t}")
                       for t in range(2)]
        next_req = req.get(PAIRS[pi + 1], 0) if pi + 1 < len(PAIRS) else len(F)
        for tci in range(len(TKC)):
            score_chunk(b, p, tci)
            # debt-based pacing: average filler per chunk-gap is exactly
            # FILL_NS, with over/under-fill carried between gaps so the
            # coarse unit costs (2.9us conv half / 2.0us AV) average out
            debt += FILL_NS
            av_popped = False
            while debt > 0:
                # one AV unit max per gap: consecutive AV epilogues
                # serialize on gpsimd-bcast/DVE-mult/out-DMA and stall
                # the pav ring two units later. F-units required by the
                # NEXT pair take priority (a req-barrier bunch at the
                # boundary stalls the first chunk on its conv bias);
                # otherwise keep an AV reserve until the final pair.
                f_due = fi < next_req
                av_ok = avq and not av_popped and not f_due and (
                    last_pair or len(avq) > 4 or avq[0][0] <= pi - 2)
                if av_ok:
                    _, bb, hh, ti = avq.popleft()
                    av_qt(bb, hh, ti, pool=pconv if last_pair else None)
                    av_popped = True
                    debt -= MM_NS * 8
                elif fi < len(F):
                    debt -= max(emit_f(), 1)
                else:
                    break
            if last_pair:
                # even head's AV contribution for this chunk
                kt0, ktn = TKC[tci]
                he = 2 * p
                for ti, (qt0, qtn) in enumerate(TQ):
                    nc.tensor.matmul(
                        pa_last[ti][:2 * DH, :qtn],
                        v_tiles[b][:ktn, tci, he, :],
                        pt_tiles[(b, he)][tci][:ktn, qt0:qt0 + qtn],
                        start=(tci == 0), stop=(tci == len(TKC) - 1),
                        skip_group_check=True,
                    )
        if last_pair:
            for ti in range(2):
                av_epilogue(pa_last[ti], b, 2 * p, ti)
            del pt_tiles[(b, 2 * p)]
            for ti in range(2):
                av_qt(b, 2 * p + 1, ti, pool=pconv)
            continue
        for h in (2 * p, 2 * p + 1):
            for ti in range(2):
                if fi >= v_done_idx[b]:
                    avq.append((pi, b, h, ti))
                else:
                    pending_av.append((pi, b, h, ti))
        promote_pending()

    # drain
    while fi < len(F):
        emit_f()
    while pending_av:
        avq.append(pending_av.popleft())
    while avq:
        _, bb, hh, ti = avq.popleft()
        av_qt(bb, hh, ti)
    ctx.close()


_CACHE = {}


def _build():
    key = "nc"
    if key in _CACHE:
        return _CACHE[key]
    nc = bacc.Bacc("TRN2", target_bir_lowering=False, debug=False,
                   num_devices=NCORES)
    xs = nc.dram_tensor("xs", [BLOC, D, L], MMDT, kind="ExternalInput").ap()
    wq = nc.dram_tensor("wqt", [128, NIC, KW, D], MMDT, kind="ExternalInput").ap()
    wk = nc.dram_tensor("wkt", [128, NIC, KW, D], MMDT, kind="ExternalInput").ap()
    wv = nc.dram_tensor("wvt", [128, NIC, KW, D], MMDT, kind="ExternalInput").ap()
    bq = nc.dram_tensor("bq", [128, NIC], F32, kind="ExternalInput").ap()
    bk = nc.dram_tensor("bk", [128, NIC], F32, kind="ExternalInput").ap()
    bv = nc.dram_tensor("bv", [128, D], F32, kind="ExternalInput").ap()
    out = nc.dram_tensor("out", [BLOC, D, LP], BF16, kind="ExternalOutput").ap()
    with tile.TileContext(nc) as tc:
        _emit(tc, xs, wq, wk, wv, bq, bk, bv, out)
    nc.compile()
    _CACHE[key] = nc
    return nc


def _wt(w):
    # w: [O, I, K] -> [p, ic, k, o] with i = ic*128 + p
    return np.ascontiguousarray(
        w.transpose(1, 2, 0).reshape(NIC, 128, KW, D).transpose(1, 0, 2, 3)
    ).astype(MMDT_NP)


def _in_common(w0, b0, w1, b1, w2, b2):
    return {
        "wqt": _wt(np.asarray(w0, np.float32)),
        "wkt": _wt(np.asarray(w1, np.float32)),
        "wvt": _wt(np.asarray(w2, np.float32)),
        # per-partition bias layouts: [p, oc] with o = oc*128+p
        "bq": np.ascontiguousarray(
            np.asarray(b0, np.float32).reshape(NIC, 128).T),
        "bk": np.ascontiguousarray(
            np.asarray(b1, np.float32).reshape(NIC, 128).T),
        "bv": np.ascontiguousarray(
            np.tile(np.asarray(b2, np.float32)[None, :], (128, 1))),
    }


def kernel(x, w0, b0, w1, b1, w2, b2):
    x = np.asarray(x, dtype=np.float32)
    inp_common = _in_common(w0, b0, w1, b1, w2, b2)
    nc = _build()
    in_maps = [
        {"xs": np.ascontiguousarray(x[c * BLOC:(c + 1) * BLOC]).astype(MMDT_NP), **inp_common}
        for c in range(NCORES)
    ]
    res = run_bass_kernel_spmd(nc, in_maps, list(range(NCORES)))
    return np.concatenate(
        [res.results[c]["out"].astype(np.float32) for c in range(NCORES)], axis=0)


def run_traced(x, w0, b0, w1, b1, w2, b2, **kw):
    """Like kernel() but returns (output, BassKernelResults)."""
    x = np.asarray(x, dtype=np.float32)
    inp_common = _in_common(w0, b0, w1, b1, w2, b2)
    nc = _build()
    in_maps = [
        {"xs": np.ascontiguousarray(x[c * BLOC:(c + 1) * BLOC]).astype(MMDT_NP), **inp_common}
        for c in range(NCORES)
    ]
    res = run_bass_kernel_spmd(nc, in_maps, list(range(NCORES)), **kw)
    out = np.concatenate(
        [res.results[c]["out"].astype(np.float32) for c in range(NCORES)], axis=0)
    return out, res


# revision 34
# speedup vs baseline: 1.0026x; 1.0026x over previous
"""Trainium2 Bass kernel: ConvolutionalMultiheadAttention.

Reference computation (per batch element b):
    q = conv1d(x, w0) + b0          # [D, Lp]  (VALID, K=3)
    k = conv1d(x, w1) + b1
    v = conv1d(x, w2) + b2
    per head h (Dh=64): out_h = v_h @ softmax(q_h^T k_h / sqrt(D))^T

Sharding: data-parallel over batch B=16 across 8 cores (2 per core).
Weights replicated. No collectives.

Measured on idle hw: ~271us (baseline ~297us). Note the chip drops to
~2.0GHz (P0 power state) under sustained back-to-back benching, which
inflates everything by ~1.2x; compare runs only at equal clock.

Per-core kernel architecture v2 (PE-roofline oriented):
  - conv as matmul: contraction over input channel i (4 chunks of 128),
    12 accumulating matmuls per PSUM tile. q,k in [o_part, t_free]
    layout; v transposed [t_part, o_free].
  - v tile layout [t_part, ktc, h, 128]: cols 0..63 constant 1.0, cols
    64..127 hold v. The AV matmul yields the softmax denominator on
    PSUM partition 0 and the output block on partitions 64..127 for
    free (matmul cost is column-streaming-bound, M-independent).
  - scores PAIRED: heads 2p (partitions 0:64 of oc=p) and 2p+1 (64:128)
    are emitted as alternating matmuls with row-tile positions (0,0)
    and (64,0) (auto-derived from base_partition). The two K=64 MMs
    occupy disjoint row groups of the PE array and run CONCURRENTLY,
    halving score stream time. exp via ACT with the 1/sqrt(512) scale
    folded in, fp16 out.
  - unified fine-grained scheduler: score chunks (4 MMs + 2 exps each)
    are emitted every ~FILL_NS of conv/AV filler work so the 2-buf
    score-PSUM ring never stalls the PE behind ACT (2x1.11us exp per
    chunk), the ACT engine starts its 142us of exp work at ~20us, and
    AV units drain the pt ring one pair behind the score stream.
  - HAM pre-warm: WARMUP dummy matmuls on a memset tile keep the PE
    busy from the end of the engine preamble (~7us) through the DMA
    load window so the conv stream starts at 2.4GHz (K=8/8) instead of
    paying the cold 1.2GHz clock until ~30us.
  - input DMA: sync issues the first-need-critical stream (wq0 kk
    slices, x0 a-halves, wk0); scalar issues x0 b-halves + remaining
    weights in parallel (each dma_start costs ~600ns of serial
    descriptor-write time on its issuing engine); gpsimd issues biases
    and the bulk wv/x1 loads gated behind the first conv output.
"""

import numpy as np

import concourse.bass as bass
import concourse.bacc as bacc
import concourse.mybir as mybir
import concourse.tile as tile
from concourse.bass_utils import run_bass_kernel_spmd

B, D, L, KW, H = 16, 512, 1024, 3, 8
LP = L - KW + 1          # 1022
DH = D // H              # 64
NCORES = 8
BLOC = B // NCORES       # 2
NIC = D // 128           # 4 input-channel chunks
SCALE = 1.0 / float(np.sqrt(D))
import os
MM_DTYPE_NAME = os.environ.get('MM_DTYPE', 'bf16')

F32 = mybir.dt.float32
F32R = mybir.dt.float32r
F16 = mybir.dt.float16
BF16 = mybir.dt.bfloat16
MMDT = {"f32r": F32R, "bf16": BF16, "f32": F32}[MM_DTYPE_NAME]
import ml_dtypes
MMDT_NP = {"f32r": np.float32, "bf16": ml_dtypes.bfloat16, "f32": np.float32}[MM_DTYPE_NAME]

# time chunking
TQ = [(0, 512), (512, LP - 512)]                       # qt chunks (512, 510)
TKC = [(i * 128, min(128, LP - i * 128)) for i in range(8)]  # kt chunks (...126)

PT_BUFS = int(os.environ.get('PT_BUFS', '32'))
WARMUP = int(os.environ.get('WARMUP', '24'))
FILL_NS = int(os.environ.get('FILL_NS', '2550'))
SCALAR_DMA = int(os.environ.get('SCALAR_DMA', '1'))

# PE-time cost model for pacing (ns): ~244ns per N=512 matmul slot
MM_NS = 244


def _emit(tc, xs, wq, wk, wv, bq, bk, bv, out):
    nc = tc.nc
    Exp = mybir.ActivationFunctionType.Exp
    from concourse.alu_op_type import AluOpType
    Add = AluOpType.add
    from contextlib import ExitStack
    ctx = ExitStack()
    wpool = ctx.enter_context(tc.tile_pool(name="w", bufs=1))
    cpool = ctx.enter_context(tc.tile_pool(name="const", bufs=1))
    xpool = ctx.enter_context(tc.tile_pool(name="x", bufs=1))
    qkpool = ctx.enter_context(tc.tile_pool(name="qk", bufs=1))
    vpool = ctx.enter_context(tc.tile_pool(name="v", bufs=2))
    ptpool = ctx.enter_context(tc.tile_pool(name="pt", bufs=PT_BUFS))
    opool = ctx.enter_context(tc.tile_pool(name="o", bufs=6))
    rpool = ctx.enter_context(tc.tile_pool(name="r", bufs=4))
    bpool = ctx.enter_context(tc.tile_pool(name="bc", bufs=4))
    # PSUM (8 banks): pscore 2x[128,1024] (4) + pconv 2x[128,512] (2)
    # + pav 2x[128,512] (2)
    pscore = ctx.enter_context(tc.tile_pool(name="pscore", bufs=2, space="PSUM"))
    pconv = ctx.enter_context(tc.tile_pool(name="pconv", bufs=2, space="PSUM"))
    pav = ctx.enter_context(tc.tile_pool(name="pav", bufs=2, space="PSUM"))

    # ---- loads: two parallel issuers (sync: first-need critical;
    # scalar: second-wave), gpsimd: biases + gated bulk ----
    wq_ic = []
    wk_ic = []
    x_t = [[None] * NIC for _ in range(BLOC)]
    x0r = xs[0].rearrange("(c p) t -> p c t", p=128)
    x1r = xs[1].rearrange("(c p) t -> p c t", p=128)

    for ic in range(NIC):
        wq_ic.append(wpool.tile([128, KW, D], MMDT, tag=f"wq{ic}", name=f"wq{ic}"))
        wk_ic.append(wpool.tile([128, KW, D], MMDT, tag=f"wk{ic}", name=f"wk{ic}"))
        # overlapping halves: xa covers cols 0..516, xb covers 512..1024,
        # so every conv matmul depends on exactly ONE x DMA
        x_t[0][ic] = (
            xpool.tile([128, 516], MMDT, tag=f"x0{ic}a", name=f"x0{ic}a"),
            xpool.tile([128, 512], MMDT, tag=f"x0{ic}b", name=f"x0{ic}b"),
        )
    # three parallel issue queues, each in first-need order:
    #   sync:   wq0 kk-slices, x0 a-halves, wk0, x0 b-halves (~1.8MB)
    #   gpsimd: biases, wq1, wk1 (~0.8MB)
    #   scalar: wq2, wk2, wq3, wk3 (late needs, ~1.6MB)
    for kk in range(KW):
        nc.sync.dma_start(wq_ic[0][:, kk], wq[:, 0, kk])
    nc.sync.dma_start(x_t[0][0][0][:], x0r[:, 0, 0:516])
    nc.sync.dma_start(x_t[0][1][0][:], x0r[:, 1, 0:516])
    nc.sync.dma_start(x_t[0][2][0][:], x0r[:, 2, 0:516])
    if SCALAR_DMA:
        nc.scalar.dma_start(x_t[0][3][0][:], x0r[:, 3, 0:516])
        nc.scalar.dma_start(wk_ic[0][:], wk[:, 0])
    else:
        nc.sync.dma_start(x_t[0][3][0][:], x0r[:, 3, 0:516])
        nc.sync.dma_start(wk_ic[0][:], wk[:, 0])
    bq_sb = cpool.tile([128, NIC], F32, tag="bq")
    nc.gpsimd.dma_start(bq_sb[:], bq[:])
    bk_sb = cpool.tile([128, NIC], F32, tag="bk")
    nc.gpsimd.dma_start(bk_sb[:], bk[:])
    for ic in range(NIC):
        nc.gpsimd.dma_start(x_t[0][ic][1][:], x0r[:, ic, 512:L])
    nc.gpsimd.dma_start(wq_ic[1][:], wq[:, 1])
    nc.gpsimd.dma_start(wk_ic[1][:], wk[:, 1])
    wv_sb = wpool.tile([128, NIC, KW, D], MMDT, tag="wv")
    bv_sb = cpool.tile([128, D], F32, tag="bv")
    for ic in range(NIC):
        x_t[1][ic] = (
            xpool.tile([128, 516], MMDT, tag=f"x1{ic}a", name=f"x1{ic}a"),
            xpool.tile([128, 512], MMDT, tag=f"x1{ic}b", name=f"x1{ic}b"),
        )

    def load_bulk_wv():
        # gate the 1.5MB wv load + second-wave weights on the first conv
        # output: the input stream is aggregate-HBM-bandwidth-bound
        # across all 8 cores, so bytes not needed until ~50us must not
        # compete with the critical wq0/wk0/x0/wq1/wk1 transfers
        gate = cpool.tile([1, 8], MMDT, tag="gate")
        from concourse.alu_op_type import AluOpType as _A
        nc.gpsimd.tensor_tensor(gate[0:1, :], q_oc[0][0:1, 0:8],
                                q_oc[0][0:1, 0:8], op=_A.add)
        nc.gpsimd.dma_start(wv_sb[:], wv[:])
        nc.gpsimd.dma_start(bv_sb[:], bv[:])
        nc.gpsimd.dma_start(wq_ic[3][:], wq[:, 3])
        nc.gpsimd.dma_start(wk_ic[3][:], wk[:, 3])
        # scalar carries oc2 in parallel, gated the same way
        sgate = cpool.tile([1, 8], F16, tag="sgate")
        nc.scalar.activation(sgate[0:1, :], q_oc[0][0:1, 0:8],
                             mybir.ActivationFunctionType.Exp)
        nc.scalar.dma_start(wq_ic[2][:], wq[:, 2])
        nc.scalar.dma_start(wk_ic[2][:], wk[:, 2])

    def load_bulk_x1():
        for ic in range(NIC):
            nc.gpsimd.dma_start(x_t[1][ic][0][:], x1r[:, ic, 0:516])
            nc.gpsimd.dma_start(x_t[1][ic][1][:], x1r[:, ic, 512:L])

    def x_slice(b, ic, g0, gn):
        xa, xb = x_t[b][ic]
        if g0 + gn <= 516:
            return xa[:, g0:g0 + gn]
        assert g0 >= 512
        return xb[:, g0 - 512:g0 - 512 + gn]

    def w_slice(nm, ic, kk, osl):
        if nm == "q":
            return wq_ic[ic][:, kk, osl]
        if nm == "k":
            return wk_ic[ic][:, kk, osl]
        return wv_sb[:, ic, kk, osl]

    # q/k: per-oc tiles [p, t] with o = oc*128+p
    q_oc = [qkpool.tile([128, L], MMDT, tag=f"q{oc}", name=f"q{oc}") for oc in range(NIC)]
    k_oc = [qkpool.tile([128, L], MMDT, tag=f"k{oc}", name=f"k{oc}") for oc in range(NIC)]
    v_tiles = [None, None]

    # ---- HAM pre-warm: dummy matmuls on a memset tile keep the PE
    # busy through the DMA-load window so real matmuls start warm ----
    wu = cpool.tile([128, 512], MMDT, tag="wu")
    nc.vector.memset(wu[:], 0.0)

    def wu_fill(n):
        for _ in range(n):
            pw = pav.tile([128, 512], F32, tag="pa", name="pw")
            nc.tensor.matmul(pw[:, :], wu[:, 0:128], wu[:, :],
                             start=True, stop=True)

    wu_fill(WARMUP)

    # ---- building blocks ----
    bias_flip = [0]

    def conv_qk_half(b, nm, oc, ti):
        t0, tn = TQ[ti]
        dst = (q_oc if nm == "q" else k_oc)[oc]
        bias_sb = bq_sb if nm == "q" else bk_sb
        ps = pconv.tile([128, 512], F32, tag="pc", name="pc")
        mm = 0
        for ic in range(NIC):
            for kk in range(KW):
                nc.tensor.matmul(
                    ps[:, :tn],
                    w_slice(nm, ic, kk, slice(oc * 128, (oc + 1) * 128)),
                    x_slice(b, ic, t0 + kk, tn),
                    start=(mm == 0), stop=(mm == NIC * KW - 1),
                )
                mm += 1
        nc.vector.tensor_tensor(
            dst[:, t0:t0 + tn], ps[:, :tn],
            bias_sb[:, oc:oc + 1].broadcast_to([128, tn]), op=Add,
        )

    def v_alloc(b):
        v_sb = vpool.tile([128, 8, H, 2 * DH], F16, tag="v")
        nc.vector.memset(v_sb[:, :, :, 0:DH], 1.0)
        v_tiles[b] = v_sb

    def conv_v_piece(b, tci):
        t0, tn = TKC[tci]
        ps = pconv.tile([128, 512], F32, tag="pc", name="pc")
        mm = 0
        for ic in range(NIC):
            for kk in range(KW):
                nc.tensor.matmul(
                    ps[:tn, :],
                    x_slice(b, ic, t0 + kk, tn),
                    wv_sb[:, ic, kk, :],
                    start=(mm == 0), stop=(mm == NIC * KW - 1),
                )
                mm += 1
        nc.vector.tensor_tensor(
            v_tiles[b][:tn, tci, :, DH:2 * DH],
            ps[:tn].rearrange("p (h d) -> p h d", h=H),
            bv_sb[:tn].rearrange("p (h d) -> p h d", h=H),
            op=Add,
        )

    # ---- attention ----
    pt_tiles = {}

    def score_chunk(b, p, tci):
        # paired scores for heads (2p, 2p+1): alternating matmuls at
        # row-tile positions (0,0)/(64,0) run concurrently in the PE
        oc = p
        kt0, ktn = TKC[tci]
        kh_e = k_oc[oc][0:DH, :]
        qh_e = q_oc[oc][0:DH, :]
        kh_o = k_oc[oc][DH:2 * DH, :]
        qh_o = q_oc[oc][DH:2 * DH, :]
        pt_e = ptpool.tile([128, L], F16, tag="pt")
        pt_o = ptpool.tile([128, L], F16, tag="pt")
        ss_e = pscore.tile([128, 1024], F32, tag="ps", name="ss")
        ss_o = pscore.tile([128, 1024], F32, tag="ps", name="ss")
        for (qt0, qtn) in TQ:
            nc.tensor.matmul(
                ss_e[:ktn, qt0:qt0 + qtn],
                kh_e[:, kt0:kt0 + ktn],
                qh_e[:, qt0:qt0 + qtn],
                start=True, stop=True,
            )
            nc.tensor.matmul(
                ss_o[:ktn, qt0:qt0 + qtn],
                kh_o[:, kt0:kt0 + ktn],
                qh_o[:, qt0:qt0 + qtn],
                start=True, stop=True,
            )
        nc.scalar.activation(pt_e[:ktn, 0:LP], ss_e[:ktn, 0:LP], Exp,
                             scale=SCALE)
        nc.scalar.activation(pt_o[:ktn, 0:LP], ss_o[:ktn, 0:LP], Exp,
                             scale=SCALE)
        pt_tiles.setdefault((b, 2 * p), []).append(pt_e)
        pt_tiles.setdefault((b, 2 * p + 1), []).append(pt_o)

    def av_epilogue(pa, b, h, ti):
        qt0, qtn = TQ[ti]
        rec = rpool.tile([1, 512], F32, tag="rec")
        nc.vector.reciprocal_approx_fast(rec[:1, :qtn], pa[0:1, :qtn])
        brd = bpool.tile([DH, 512], F32, tag="brd")
        nc.gpsimd.partition_broadcast(brd[:, :qtn], rec[:1, :qtn])
        ot = opool.tile([DH, 512], BF16, tag="ot")
        nc.vector.tensor_mul(ot[:, :qtn], pa[DH:2 * DH, :qtn], brd[:, :qtn])
        nc.sync.dma_start(
            out[b, DH * h:DH * (h + 1), qt0:qt0 + qtn], ot[:, :qtn]
        )

    def av_qt(b, h, ti, pool=None):
        qt0, qtn = TQ[ti]
        tiles = pt_tiles[(b, h)]
        # tag must match the pool's existing ring ("pc" in pconv) or the
        # pool footprint doubles and PSUM overflows
        pa = (pconv.tile([128, 512], F32, tag="pc", name="pc")
              if pool is pconv else
              pav.tile([128, 512], F32, tag="pa", name="pa"))
        for tci, (kt0, ktn) in enumerate(TKC):
            nc.tensor.matmul(
                pa[:2 * DH, :qtn],
                v_tiles[b][:ktn, tci, h, :],
                tiles[tci][:ktn, qt0:qt0 + qtn],
                start=(tci == 0), stop=(tci == len(TKC) - 1),
            )
        av_epilogue(pa, b, h, ti)
        if ti == 1:
            del pt_tiles[(b, h)]

    # ---- planner: filler units with costs, pair-prereq barriers ----
    def qk_unit(b, nm, oc, ti):
        return (MM_NS * 12, lambda: conv_qk_half(b, nm, oc, ti))

    def v_unit(b, tci):
        return (MM_NS * 12, lambda: conv_v_piece(b, tci))

    def misc_unit(fn):
        return (0, fn)

    F = []
    req = {}  # pair -> required F prefix length
    # b0: v pieces early (so pair-0 AVs can flow during pair 1), then
    # remaining q/k convs; wv/x1 bulk-load triggers first
    F += [qk_unit(0, "q", 1, 0), qk_unit(0, "q", 1, 1),
          misc_unit(load_bulk_wv),
          qk_unit(0, "k", 1, 0), qk_unit(0, "k", 1, 1)]
    req[(0, 1)] = len(F)
    F += [misc_unit(load_bulk_x1), misc_unit(lambda: v_alloc(0)),
          v_unit(0, 0), v_unit(0, 1), v_unit(0, 2), v_unit(0, 3),
          v_unit(0, 4), v_unit(0, 5), v_unit(0, 6), v_unit(0, 7)]
    v_done_idx = {0: len(F)}
    F += [qk_unit(0, "q", 2, 0), qk_unit(0, "q", 2, 1),
          qk_unit(0, "k", 2, 0), qk_unit(0, "k", 2, 1)]
    req[(0, 2)] = len(F)
    F += [qk_unit(0, "q", 3, 0), qk_unit(0, "q", 3, 1),
          qk_unit(0, "k", 3, 0), qk_unit(0, "k", 3, 1)]
    req[(0, 3)] = len(F)
    F += [qk_unit(1, "q", 0, 0), qk_unit(1, "q", 0, 1),
          qk_unit(1, "k", 0, 0), qk_unit(1, "k", 0, 1)]
    req[(1, 0)] = len(F)
    F += [misc_unit(lambda: v_alloc(1)),
          v_unit(1, 0), v_unit(1, 1), v_unit(1, 2), v_unit(1, 3),
          v_unit(1, 4), v_unit(1, 5), v_unit(1, 6), v_unit(1, 7)]
    v_done_idx[1] = len(F)
    F += [qk_unit(1, "q", 1, 0), qk_unit(1, "q", 1, 1),
          qk_unit(1, "k", 1, 0), qk_unit(1, "k", 1, 1)]
    req[(1, 1)] = len(F)
    F += [qk_unit(1, "q", 2, 0), qk_unit(1, "q", 2, 1),
          qk_unit(1, "k", 2, 0), qk_unit(1, "k", 2, 1)]
    req[(1, 2)] = len(F)
    F += [qk_unit(1, "q", 3, 0), qk_unit(1, "q", 3, 1),
          qk_unit(1, "k", 3, 0), qk_unit(1, "k", 3, 1)]
    req[(1, 3)] = len(F)

    # prelude: b0 q0/k0 conv (required by pair (0,0)); t0 pieces first
    # (they only need the x a-halves, which arrive before the b-halves).
    # Warmup matmuls interleave so DMA-arrival stalls inside the prelude
    # never idle the PE past the ~3.4us HAM re-throttle window.
    conv_qk_half(0, "q", 0, 0)
    wu_fill(4)
    conv_qk_half(0, "k", 0, 0)
    wu_fill(4)
    conv_qk_half(0, "q", 0, 1)
    wu_fill(4)
    conv_qk_half(0, "k", 0, 1)

    PAIRS = [(0, 0), (0, 1), (0, 2), (0, 3),
             (1, 0), (1, 1), (1, 2), (1, 3)]
    from collections import deque
    avq = deque()          # entries: (pair_idx, b, h, ti)
    pending_av = deque()
    fi = 0
    debt = 0

    def promote_pending():
        # move avs whose batch's v conv is fully emitted into the queue
        while pending_av and fi >= v_done_idx[pending_av[0][1]]:
            avq.append(pending_av.popleft())

    def emit_f():
        nonlocal fi
        cost, fn = F[fi]
        fn()
        fi += 1
        promote_pending()
        return cost

    for pi, (b, p) in enumerate(PAIRS):
        # pt-ring safety: pair pi's chunks reuse the pt bufs of pair
        # pi-2; those readers (pair pi-2's AV matmuls) MUST already be
        # in the PE stream or the ring-writer wait deadlocks. Interleave
        # the forced AVs with the required conv units so consecutive AV
        # epilogues don't serialize on gpsimd-bcast/mult/out-DMA.
        while fi < req.get((b, p), 0) or (avq and avq[0][0] <= pi - 2):
            if fi < req.get((b, p), 0):
                emit_f()
            if avq and avq[0][0] <= pi - 2:
                _, bb, hh, ti = avq.popleft()
                av_qt(bb, hh, ti)
        assert not any(e[0] <= pi - 2 for e in pending_av), (
            f"pt-ring deadlock: pair {pi} needs avs of pair {pi-2} "
            f"but v conv not yet emitted")
        last_pair = pi == len(PAIRS) - 1
        if last_pair:
            # final pair: the even head's AV accumulation groups stay
            # OPEN in the pav banks and receive one matmul per score
            # chunk as its exp lands, instead of waiting for the whole
            # pair. Reserve AVs drain through the (now idle) pconv
            # banks. This fills the ACT-paced terminal chunk-gaps and
            # shortens the post-exp drain.
            pa_last = [pav.tile([128, 512], F32, tag="pa", name=f"paL{t}")
                       for t in range(2)]
        next_req = req.get(PAIRS[pi + 1], 0) if pi + 1 < len(PAIRS) else len(F)
        for tci in range(len(TKC)):
            score_chunk(b, p, tci)
            # debt-based pacing: average filler per chunk-gap is exactly
            # FILL_NS, with over/under-fill carried between gaps so the
            # coarse unit costs (2.9us conv half / 2.0us AV) average out
            debt += FILL_NS
            av_popped = False
            while debt > 0:
                # one AV unit max per gap: consecutive AV epilogues
                # serialize on gpsimd-bcast/DVE-mult/out-DMA and stall
                # the pav ring two units later. F-units required by the
                # NEXT pair take priority (a req-barrier bunch at the
                # boundary stalls the first chunk on its conv bias);
                # otherwise keep an AV reserve until the final pair.
                f_due = fi < next_req
                av_ok = avq and not av_popped and not f_due and (
                    last_pair or len(avq) > 4 or avq[0][0] <= pi - 2)
                if av_ok:
                    _, bb, hh, ti = avq.popleft()
                    av_qt(bb, hh, ti, pool=pconv if last_pair else None)
                    av_popped = True
                    debt -= MM_NS * 8
                elif fi < len(F):
                    debt -= max(emit_f(), 1)
                else:
                    break
            if last_pair:
                # even head's AV contribution for this chunk
                kt0, ktn = TKC[tci]
                he = 2 * p
                for ti, (qt0, qtn) in enumerate(TQ):
                    nc.tensor.matmul(
                        pa_last[ti][:2 * DH, :qtn],
                        v_tiles[b][:ktn, tci, he, :],
                        pt_tiles[(b, he)][tci][:ktn, qt0:qt0 + qtn],
                        start=(tci == 0), stop=(tci == len(TKC) - 1),
                        skip_group_check=True,
                    )
        if last_pair:
            for ti in range(2):
                av_epilogue(pa_last[ti], b, 2 * p, ti)
            del pt_tiles[(b, 2 * p)]
            for ti in range(2):
                av_qt(b, 2 * p + 1, ti, pool=pconv)
            continue
        for h in (2 * p, 2 * p + 1):
            for ti in range(2):
                if fi >= v_done_idx[b]:
                    avq.append((pi, b, h, ti))
                else:
                    pending_av.append((pi, b, h, ti))
        promote_pending()

    # drain
    while fi < len(F):
        emit_f()
    while pending_av:
        avq.append(pending_av.popleft())
    while avq:
        _, bb, hh, ti = avq.popleft()
        av_qt(bb, hh, ti)
    ctx.close()


_CACHE = {}


def _build():
    key = "nc"
    if key in _CACHE:
        return _CACHE[key]
    nc = bacc.Bacc("TRN2", target_bir_lowering=False, debug=False,
                   num_devices=NCORES)
    xs = nc.dram_tensor("xs", [BLOC, D, L], MMDT, kind="ExternalInput").ap()
    wq = nc.dram_tensor("wqt", [128, NIC, KW, D], MMDT, kind="ExternalInput").ap()
    wk = nc.dram_tensor("wkt", [128, NIC, KW, D], MMDT, kind="ExternalInput").ap()
    wv = nc.dram_tensor("wvt", [128, NIC, KW, D], MMDT, kind="ExternalInput").ap()
    bq = nc.dram_tensor("bq", [128, NIC], F32, kind="ExternalInput").ap()
    bk = nc.dram_tensor("bk", [128, NIC], F32, kind="ExternalInput").ap()
    bv = nc.dram_tensor("bv", [128, D], F32, kind="ExternalInput").ap()
    out = nc.dram_tensor("out", [BLOC, D, LP], BF16, kind="ExternalOutput").ap()
    with tile.TileContext(nc) as tc:
        _emit(tc, xs, wq, wk, wv, bq, bk, bv, out)
    nc.compile()
    _CACHE[key] = nc
    return nc


def _wt(w):
    # w: [O, I, K] -> [p, ic, k, o] with i = ic*128 + p
    return np.ascontiguousarray(
        w.transpose(1, 2, 0).reshape(NIC, 128, KW, D).transpose(1, 0, 2, 3)
    ).astype(MMDT_NP)


def _in_common(w0, b0, w1, b1, w2, b2):
    return {
        "wqt": _wt(np.asarray(w0, np.float32)),
        "wkt": _wt(np.asarray(w1, np.float32)),
        "wvt": _wt(np.asarray(w2, np.float32)),
        # per-partition bias layouts: [p, oc] with o = oc*128+p
        "bq": np.ascontiguousarray(
            np.asarray(b0, np.float32).reshape(NIC, 128).T),
        "bk": np.ascontiguousarray(
            np.asarray(b1, np.float32).reshape(NIC, 128).T),
        "bv": np.ascontiguousarray(
            np.tile(np.asarray(b2, np.float32)[None, :], (128, 1))),
    }


def kernel(x, w0, b0, w1, b1, w2, b2):
    x = np.asarray(x, dtype=np.float32)
    inp_common = _in_common(w0, b0, w1, b1, w2, b2)
    nc = _build()
    in_maps = [
        {"xs": np.ascontiguousarray(x[c * BLOC:(c + 1) * BLOC]).astype(MMDT_NP), **inp_common}
        for c in range(NCORES)
    ]
    res = run_bass_kernel_spmd(nc, in_maps, list(range(NCORES)))
    return np.concatenate(
        [res.results[c]["out"].astype(np.float32) for c in range(NCORES)], axis=0)


def run_traced(x, w0, b0, w1, b1, w2, b2, **kw):
    """Like kernel() but returns (output, BassKernelResults)."""
    x = np.asarray(x, dtype=np.float32)
    inp_common = _in_common(w0, b0, w1, b1, w2, b2)
    nc = _build()
    in_maps = [
        {"xs": np.ascontiguousarray(x[c * BLOC:(c + 1) * BLOC]).astype(MMDT_NP), **inp_common}
        for c in range(NCORES)
    ]
    res = run_bass_kernel_spmd(nc, in_maps, list(range(NCORES)), **kw)
    out = np.concatenate(
        [res.results[c]["out"].astype(np.float32) for c in range(NCORES)], axis=0)
    return out, res


# revision 49
# speedup vs baseline: 1.0050x; 1.0025x over previous
"""Trainium2 Bass kernel: ConvolutionalMultiheadAttention.

Reference computation (per batch element b):
    q = conv1d(x, w0) + b0          # [D, Lp]  (VALID, K=3)
    k = conv1d(x, w1) + b1
    v = conv1d(x, w2) + b2
    per head h (Dh=64): out_h = v_h @ softmax(q_h^T k_h / sqrt(D))^T

Sharding: data-parallel over batch B=16 across 8 cores (2 per core).
Weights replicated. No collectives.

Measured on idle hw: ~270.7us (baseline ~297.5us), rel err 4.7e-3. Note
the chip drops to ~2.0GHz (P0 power state) under sustained back-to-back
benching, inflating everything ~1.2x; compare runs only at equal clock.
Correctness is validated on BOTH the traced and untraced execution
paths (they exercise different timing; see load_second_wave comment).
q/k weights are staged host-side into oc-major tiles so the first score
pair gates on 0.8MB instead of 3.1MB of weights.

Per-core kernel architecture v2 (PE-roofline oriented):
  - conv as matmul: contraction over input channel i (4 chunks of 128),
    12 accumulating matmuls per PSUM tile. q,k in [o_part, t_free]
    layout; v transposed [t_part, o_free].
  - v tile layout [t_part, ktc, h, 128]: cols 0..63 constant 1.0, cols
    64..127 hold v. The AV matmul yields the softmax denominator on
    PSUM partition 0 and the output block on partitions 64..127 for
    free (matmul cost is column-streaming-bound, M-independent).
  - scores PAIRED: heads 2p (partitions 0:64 of oc=p) and 2p+1 (64:128)
    are emitted as alternating matmuls with row-tile positions (0,0)
    and (64,0) (auto-derived from base_partition). The two K=64 MMs
    occupy disjoint row groups of the PE array and run CONCURRENTLY,
    halving score stream time. exp via ACT with the 1/sqrt(512) scale
    folded in, fp16 out.
  - unified fine-grained scheduler: score chunks (4 MMs + 2 exps each)
    are emitted every ~FILL_NS of conv/AV filler work so the 2-buf
    score-PSUM ring never stalls the PE behind ACT (2x1.11us exp per
    chunk), the ACT engine starts its 142us of exp work at ~20us, and
    AV units drain the pt ring one pair behind the score stream.
  - HAM pre-warm: WARMUP dummy matmuls on a memset tile keep the PE
    busy from the end of the engine preamble (~7us) through the DMA
    load window so the conv stream starts at 2.4GHz (K=8/8) instead of
    paying the cold 1.2GHz clock until ~30us.
  - input DMA: sync issues the first-need-critical stream (wq0 kk
    slices, x0 a-halves, wk0); scalar issues x0 b-halves + remaining
    weights in parallel (each dma_start costs ~600ns of serial
    descriptor-write time on its issuing engine); gpsimd issues biases
    and the bulk wv/x1 loads gated behind the first conv output.
"""

import numpy as np

import concourse.bass as bass
import concourse.bacc as bacc
import concourse.mybir as mybir
import concourse.tile as tile
from concourse.bass_utils import run_bass_kernel_spmd

B, D, L, KW, H = 16, 512, 1024, 3, 8
LP = L - KW + 1          # 1022
DH = D // H              # 64
NCORES = 8
BLOC = B // NCORES       # 2
NIC = D // 128           # 4 input-channel chunks
SCALE = 1.0 / float(np.sqrt(D))
import os
MM_DTYPE_NAME = os.environ.get('MM_DTYPE', 'bf16')

F32 = mybir.dt.float32
F32R = mybir.dt.float32r
F16 = mybir.dt.float16
BF16 = mybir.dt.bfloat16
MMDT = {"f32r": F32R, "bf16": BF16, "f32": F32}[MM_DTYPE_NAME]
import ml_dtypes
MMDT_NP = {"f32r": np.float32, "bf16": ml_dtypes.bfloat16, "f32": np.float32}[MM_DTYPE_NAME]

# time chunking
TQ = [(0, 512), (512, LP - 512)]                       # qt chunks (512, 510)
TKC = [(i * 128, min(128, LP - i * 128)) for i in range(8)]  # kt chunks (...126)

PT_BUFS = int(os.environ.get('PT_BUFS', '32'))
WARMUP = int(os.environ.get('WARMUP', '24'))
FILL_NS = int(os.environ.get('FILL_NS', '2550'))
SCALAR_DMA = int(os.environ.get('SCALAR_DMA', '1'))

# PE-time cost model for pacing (ns): ~244ns per N=512 matmul slot
MM_NS = 244


def _emit(tc, xs, wq, wk, wv, bq, bk, bv, out):
    nc = tc.nc
    Exp = mybir.ActivationFunctionType.Exp
    from concourse.alu_op_type import AluOpType
    Add = AluOpType.add
    from contextlib import ExitStack
    ctx = ExitStack()
    wpool = ctx.enter_context(tc.tile_pool(name="w", bufs=1))
    cpool = ctx.enter_context(tc.tile_pool(name="const", bufs=1))
    xpool = ctx.enter_context(tc.tile_pool(name="x", bufs=1))
    qkpool = ctx.enter_context(tc.tile_pool(name="qk", bufs=1))
    vpool = ctx.enter_context(tc.tile_pool(name="v", bufs=2))
    ptpool = ctx.enter_context(tc.tile_pool(name="pt", bufs=PT_BUFS))
    opool = ctx.enter_context(tc.tile_pool(name="o", bufs=6))
    rpool = ctx.enter_context(tc.tile_pool(name="r", bufs=4))
    bpool = ctx.enter_context(tc.tile_pool(name="bc", bufs=4))
    # PSUM (8 banks): pscore 2x[128,1024] (4) + pconv 2x[128,512] (2)
    # + pav 2x[128,512] (2)
    pscore = ctx.enter_context(tc.tile_pool(name="pscore", bufs=2, space="PSUM"))
    pconv = ctx.enter_context(tc.tile_pool(name="pconv", bufs=2, space="PSUM"))
    pav = ctx.enter_context(tc.tile_pool(name="pav", bufs=2, space="PSUM"))

    # ---- loads: two parallel issuers (sync: first-need critical;
    # scalar: second-wave), gpsimd: biases + gated bulk ----
    # wq/wk arrive in OC-MAJOR tiles: pair (0,0) then only gates on the
    # oc0 tiles (0.8MB) instead of all four ic-major tiles (3.1MB) —
    # the oc2/oc3 weights become genuinely late-need and defer by queue
    # order without any gating cycle.
    wq_oc = [wpool.tile([128, NIC, KW, 128], MMDT, tag=f"wq{oc}",
                        name=f"wq{oc}") for oc in range(NIC)]
    wk_oc = [wpool.tile([128, NIC, KW, 128], MMDT, tag=f"wk{oc}",
                        name=f"wk{oc}") for oc in range(NIC)]
    x_t = [[None] * NIC for _ in range(BLOC)]
    x0r = xs[0].rearrange("(c p) t -> p c t", p=128)
    x1r = xs[1].rearrange("(c p) t -> p c t", p=128)

    for ic in range(NIC):
        # overlapping halves: xa covers cols 0..516, xb covers 512..1024,
        # so every conv matmul depends on exactly ONE x DMA
        x_t[0][ic] = (
            xpool.tile([128, 516], MMDT, tag=f"x0{ic}a", name=f"x0{ic}a"),
            xpool.tile([128, 512], MMDT, tag=f"x0{ic}b", name=f"x0{ic}b"),
        )
    # three parallel issue queues, each in first-need order
    nc.sync.dma_start(wq_oc[0][:], wq[:, 0])
    nc.sync.dma_start(x_t[0][0][0][:], x0r[:, 0, 0:516])
    nc.sync.dma_start(x_t[0][1][0][:], x0r[:, 1, 0:516])
    nc.sync.dma_start(x_t[0][2][0][:], x0r[:, 2, 0:516])
    if SCALAR_DMA:
        nc.scalar.dma_start(x_t[0][3][0][:], x0r[:, 3, 0:516])
        nc.scalar.dma_start(wk_oc[0][:], wk[:, 0])
    else:
        nc.sync.dma_start(x_t[0][3][0][:], x0r[:, 3, 0:516])
        nc.sync.dma_start(wk_oc[0][:], wk[:, 0])
    bq_sb = cpool.tile([128, NIC], F32, tag="bq")
    nc.gpsimd.dma_start(bq_sb[:], bq[:])
    bk_sb = cpool.tile([128, NIC], F32, tag="bk")
    nc.gpsimd.dma_start(bk_sb[:], bk[:])
    for ic in range(NIC):
        nc.gpsimd.dma_start(x_t[0][ic][1][:], x0r[:, ic, 512:L])
    nc.gpsimd.dma_start(wq_oc[1][:], wq[:, 1])
    nc.gpsimd.dma_start(wk_oc[1][:], wk[:, 1])
    wv_sb = wpool.tile([128, NIC, KW, D], MMDT, tag="wv")
    bv_sb = cpool.tile([128, D], F32, tag="bv")
    for ic in range(NIC):
        x_t[1][ic] = (
            xpool.tile([128, 516], MMDT, tag=f"x1{ic}a", name=f"x1{ic}a"),
            xpool.tile([128, 512], MMDT, tag=f"x1{ic}b", name=f"x1{ic}b"),
        )

    def load_second_wave():
        # second-wave weights (oc2/oc3 of wq/wk): the input stream is
        # aggregate-HBM-bandwidth-bound across all 8 cores, so defer
        # these behind the critical wq0/x0a/wk0/x0b transfers. The gate
        # reads an x tile (a pure DMA dependency) — it must NOT read a
        # conv output: the prelude conv contracts over ic2/ic3 and so
        # depends on these very weights (a conv-output gate is a cycle
        # that only "resolves" through unsound DMA-sem fan-out aliasing,
        # returning garbage on some timing paths).
        # per-queue transfer order is the deferral mechanism: these
        # issue immediately but drain after each queue's critical set
        nc.gpsimd.dma_start(wq_oc[3][:], wq[:, 3])
        nc.gpsimd.dma_start(wk_oc[3][:], wk[:, 3])
        nc.scalar.dma_start(wq_oc[2][:], wq[:, 2])
        nc.scalar.dma_start(wk_oc[2][:], wk[:, 2])

    def load_bulk_wv():
        # wv feeds only the v conv — gating it on the first conv output
        # is acyclic and defers its 1.5MB until the q/k stream drained
        gate = cpool.tile([1, 8], MMDT, tag="wvgate")
        from concourse.alu_op_type import AluOpType as _A
        nc.gpsimd.tensor_tensor(gate[0:1, :], q_oc[0][0:1, 0:8],
                                q_oc[0][0:1, 0:8], op=_A.add)
        nc.gpsimd.dma_start(wv_sb[:], wv[:])
        nc.gpsimd.dma_start(bv_sb[:], bv[:])

    def load_bulk_x1():
        for ic in range(NIC):
            nc.gpsimd.dma_start(x_t[1][ic][0][:], x1r[:, ic, 0:516])
            nc.gpsimd.dma_start(x_t[1][ic][1][:], x1r[:, ic, 512:L])

    def x_slice(b, ic, g0, gn):
        xa, xb = x_t[b][ic]
        if g0 + gn <= 516:
            return xa[:, g0:g0 + gn]
        assert g0 >= 512
        return xb[:, g0 - 512:g0 - 512 + gn]

    def w_slice(nm, oc, ic, kk):
        if nm == "q":
            return wq_oc[oc][:, ic, kk, :]
        return wk_oc[oc][:, ic, kk, :]

    # q/k: per-oc tiles [p, t] with o = oc*128+p
    q_oc = [qkpool.tile([128, L], MMDT, tag=f"q{oc}", name=f"q{oc}") for oc in range(NIC)]
    k_oc = [qkpool.tile([128, L], MMDT, tag=f"k{oc}", name=f"k{oc}") for oc in range(NIC)]
    v_tiles = [None, None]

    load_second_wave()

    # ---- HAM pre-warm: dummy matmuls on a memset tile keep the PE
    # busy through the DMA-load window so real matmuls start warm ----
    wu = cpool.tile([128, 512], MMDT, tag="wu")
    nc.vector.memset(wu[:], 0.0)

    def wu_fill(n):
        for _ in range(n):
            pw = pav.tile([128, 512], F32, tag="pa", name="pw")
            nc.tensor.matmul(pw[:, :], wu[:, 0:128], wu[:, :],
                             start=True, stop=True)

    wu_fill(WARMUP)

    # ---- building blocks ----
    bias_flip = [0]

    def conv_qk_half(b, nm, oc, ti):
        t0, tn = TQ[ti]
        dst = (q_oc if nm == "q" else k_oc)[oc]
        bias_sb = bq_sb if nm == "q" else bk_sb
        ps = pconv.tile([128, 512], F32, tag="pc", name="pc")
        mm = 0
        for ic in range(NIC):
            for kk in range(KW):
                nc.tensor.matmul(
                    ps[:, :tn],
                    w_slice(nm, oc, ic, kk),
                    x_slice(b, ic, t0 + kk, tn),
                    start=(mm == 0), stop=(mm == NIC * KW - 1),
                )
                mm += 1
        nc.vector.tensor_tensor(
            dst[:, t0:t0 + tn], ps[:, :tn],
            bias_sb[:, oc:oc + 1].broadcast_to([128, tn]), op=Add,
        )

    def v_alloc(b):
        v_sb = vpool.tile([128, 8, H, 2 * DH], F16, tag="v")
        nc.vector.memset(v_sb[:, :, :, 0:DH], 1.0)
        v_tiles[b] = v_sb

    def conv_v_piece(b, tci):
        t0, tn = TKC[tci]
        ps = pconv.tile([128, 512], F32, tag="pc", name="pc")
        mm = 0
        for ic in range(NIC):
            for kk in range(KW):
                nc.tensor.matmul(
                    ps[:tn, :],
                    x_slice(b, ic, t0 + kk, tn),
                    wv_sb[:, ic, kk, :],
                    start=(mm == 0), stop=(mm == NIC * KW - 1),
                )
                mm += 1
        nc.vector.tensor_tensor(
            v_tiles[b][:tn, tci, :, DH:2 * DH],
            ps[:tn].rearrange("p (h d) -> p h d", h=H),
            bv_sb[:tn].rearrange("p (h d) -> p h d", h=H),
            op=Add,
        )

    # ---- attention ----
    pt_tiles = {}

    def score_chunk(b, p, tci):
        # paired scores for heads (2p, 2p+1): alternating matmuls at
        # row-tile positions (0,0)/(64,0) run concurrently in the PE
        oc = p
        kt0, ktn = TKC[tci]
        kh_e = k_oc[oc][0:DH, :]
        qh_e = q_oc[oc][0:DH, :]
        kh_o = k_oc[oc][DH:2 * DH, :]
        qh_o = q_oc[oc][DH:2 * DH, :]
        pt_e = ptpool.tile([128, L], F16, tag="pt")
        pt_o = ptpool.tile([128, L], F16, tag="pt")
        ss_e = pscore.tile([128, 1024], F32, tag="ps", name="ss")
        ss_o = pscore.tile([128, 1024], F32, tag="ps", name="ss")
        for (qt0, qtn) in TQ:
            nc.tensor.matmul(
                ss_e[:ktn, qt0:qt0 + qtn],
                kh_e[:, kt0:kt0 + ktn],
                qh_e[:, qt0:qt0 + qtn],
                start=True, stop=True,
            )
            nc.tensor.matmul(
                ss_o[:ktn, qt0:qt0 + qtn],
                kh_o[:, kt0:kt0 + ktn],
                qh_o[:, qt0:qt0 + qtn],
                start=True, stop=True,
            )
        nc.scalar.activation(pt_e[:ktn, 0:LP], ss_e[:ktn, 0:LP], Exp,
                             scale=SCALE)
        nc.scalar.activation(pt_o[:ktn, 0:LP], ss_o[:ktn, 0:LP], Exp,
                             scale=SCALE)
        pt_tiles.setdefault((b, 2 * p), []).append(pt_e)
        pt_tiles.setdefault((b, 2 * p + 1), []).append(pt_o)

    def av_epilogue(pa, b, h, ti):
        qt0, qtn = TQ[ti]
        rec = rpool.tile([1, 512], F32, tag="rec")
        nc.vector.reciprocal_approx_fast(rec[:1, :qtn], pa[0:1, :qtn])
        brd = bpool.tile([DH, 512], F32, tag="brd")
        nc.gpsimd.partition_broadcast(brd[:, :qtn], rec[:1, :qtn])
        ot = opool.tile([DH, 512], BF16, tag="ot")
        nc.vector.tensor_mul(ot[:, :qtn], pa[DH:2 * DH, :qtn], brd[:, :qtn])
        nc.sync.dma_start(
            out[b, DH * h:DH * (h + 1), qt0:qt0 + qtn], ot[:, :qtn]
        )

    def av_qt(b, h, ti, pool=None):
        qt0, qtn = TQ[ti]
        tiles = pt_tiles[(b, h)]
        # tag must match the pool's existing ring ("pc" in pconv) or the
        # pool footprint doubles and PSUM overflows
        pa = (pconv.tile([128, 512], F32, tag="pc", name="pc")
              if pool is pconv else
              pav.tile([128, 512], F32, tag="pa", name="pa"))
        for tci, (kt0, ktn) in enumerate(TKC):
            nc.tensor.matmul(
                pa[:2 * DH, :qtn],
                v_tiles[b][:ktn, tci, h, :],
                tiles[tci][:ktn, qt0:qt0 + qtn],
                start=(tci == 0), stop=(tci == len(TKC) - 1),
            )
        av_epilogue(pa, b, h, ti)
        if ti == 1:
            del pt_tiles[(b, h)]

    # ---- planner: filler units with costs, pair-prereq barriers ----
    def qk_unit(b, nm, oc, ti):
        return (MM_NS * 12, lambda: conv_qk_half(b, nm, oc, ti))

    def v_unit(b, tci):
        return (MM_NS * 12, lambda: conv_v_piece(b, tci))

    def misc_unit(fn):
        return (0, fn)

    F = []
    req = {}  # pair -> required F prefix length
    # b0: v pieces early (so pair-0 AVs can flow during pair 1), then
    # remaining q/k convs; wv/x1 bulk-load triggers first
    F += [qk_unit(0, "q", 1, 0), qk_unit(0, "q", 1, 1),
          misc_unit(load_bulk_wv),
          qk_unit(0, "k", 1, 0), qk_unit(0, "k", 1, 1)]
    req[(0, 1)] = len(F)
    F += [misc_unit(load_bulk_x1), misc_unit(lambda: v_alloc(0)),
          v_unit(0, 0), v_unit(0, 1), v_unit(0, 2), v_unit(0, 3),
          v_unit(0, 4), v_unit(0, 5), v_unit(0, 6), v_unit(0, 7)]
    v_done_idx = {0: len(F)}
    F += [qk_unit(0, "q", 2, 0), qk_unit(0, "q", 2, 1),
          qk_unit(0, "k", 2, 0), qk_unit(0, "k", 2, 1)]
    req[(0, 2)] = len(F)
    F += [qk_unit(0, "q", 3, 0), qk_unit(0, "q", 3, 1),
          qk_unit(0, "k", 3, 0), qk_unit(0, "k", 3, 1)]
    req[(0, 3)] = len(F)
    F += [qk_unit(1, "q", 0, 0), qk_unit(1, "q", 0, 1),
          qk_unit(1, "k", 0, 0), qk_unit(1, "k", 0, 1)]
    req[(1, 0)] = len(F)
    F += [misc_unit(lambda: v_alloc(1)),
          v_unit(1, 0), v_unit(1, 1), v_unit(1, 2), v_unit(1, 3),
          v_unit(1, 4), v_unit(1, 5), v_unit(1, 6), v_unit(1, 7)]
    v_done_idx[1] = len(F)
    F += [qk_unit(1, "q", 1, 0), qk_unit(1, "q", 1, 1),
          qk_unit(1, "k", 1, 0), qk_unit(1, "k", 1, 1)]
    req[(1, 1)] = len(F)
    F += [qk_unit(1, "q", 2, 0), qk_unit(1, "q", 2, 1),
          qk_unit(1, "k", 2, 0), qk_unit(1, "k", 2, 1)]
    req[(1, 2)] = len(F)
    F += [qk_unit(1, "q", 3, 0), qk_unit(1, "q", 3, 1),
          qk_unit(1, "k", 3, 0), qk_unit(1, "k", 3, 1)]
    req[(1, 3)] = len(F)

    # prelude: b0 q0/k0 conv (required by pair (0,0)); t0 pieces first
    # (they only need the x a-halves, which arrive before the b-halves).
    # Warmup matmuls interleave so DMA-arrival stalls inside the prelude
    # never idle the PE past the ~3.4us HAM re-throttle window.
    conv_qk_half(0, "q", 0, 0)
    wu_fill(4)
    conv_qk_half(0, "k", 0, 0)
    wu_fill(4)
    conv_qk_half(0, "q", 0, 1)
    wu_fill(4)
    conv_qk_half(0, "k", 0, 1)

    PAIRS = [(0, 0), (0, 1), (0, 2), (0, 3),
             (1, 0), (1, 1), (1, 2), (1, 3)]
    from collections import deque
    avq = deque()          # entries: (pair_idx, b, h, ti)
    pending_av = deque()
    fi = 0
    debt = 0

    def promote_pending():
        # move avs whose batch's v conv is fully emitted into the queue
        while pending_av and fi >= v_done_idx[pending_av[0][1]]:
            avq.append(pending_av.popleft())

    def emit_f():
        nonlocal fi
        cost, fn = F[fi]
        fn()
        fi += 1
        promote_pending()
        return cost

    for pi, (b, p) in enumerate(PAIRS):
        # pt-ring safety: pair pi's chunks reuse the pt bufs of pair
        # pi-2; those readers (pair pi-2's AV matmuls) MUST already be
        # in the PE stream or the ring-writer wait deadlocks. Interleave
        # the forced AVs with the required conv units so consecutive AV
        # epilogues don't serialize on gpsimd-bcast/mult/out-DMA.
        while fi < req.get((b, p), 0) or (avq and avq[0][0] <= pi - 2):
            if fi < req.get((b, p), 0):
                emit_f()
            if avq and avq[0][0] <= pi - 2:
                _, bb, hh, ti = avq.popleft()
                av_qt(bb, hh, ti)
        assert not any(e[0] <= pi - 2 for e in pending_av), (
            f"pt-ring deadlock: pair {pi} needs avs of pair {pi-2} "
            f"but v conv not yet emitted")
        last_pair = pi == len(PAIRS) - 1
        if last_pair:
            # final pair: the even head's AV accumulation groups stay
            # OPEN in the pav banks and receive one matmul per score
            # chunk as its exp lands, instead of waiting for the whole
            # pair. Reserve AVs drain through the (now idle) pconv
            # banks. This fills the ACT-paced terminal chunk-gaps and
            # shortens the post-exp drain.
            pa_last = [pav.tile([128, 512], F32, tag="pa", name=f"paL{t}")
                       for t in range(2)]
        next_req = req.get(PAIRS[pi + 1], 0) if pi + 1 < len(PAIRS) else len(F)
        for tci in range(len(TKC)):
            score_chunk(b, p, tci)
            # debt-based pacing: average filler per chunk-gap is exactly
            # FILL_NS, with over/under-fill carried between gaps so the
            # coarse unit costs (2.9us conv half / 2.0us AV) average out
            debt += FILL_NS
            av_popped = False
            while debt > 0:
                # one AV unit max per gap: consecutive AV epilogues
                # serialize on gpsimd-bcast/DVE-mult/out-DMA and stall
                # the pav ring two units later. F-units required by the
                # NEXT pair take priority (a req-barrier bunch at the
                # boundary stalls the first chunk on its conv bias);
                # otherwise keep an AV reserve until the final pair.
                f_due = fi < next_req
                av_ok = avq and not av_popped and not f_due and (
                    last_pair or len(avq) > 4 or avq[0][0] <= pi - 2)
                if av_ok:
                    _, bb, hh, ti = avq.popleft()
                    av_qt(bb, hh, ti, pool=pconv if last_pair else None)
                    av_popped = True
                    debt -= MM_NS * 8
                elif fi < len(F):
                    debt -= max(emit_f(), 1)
                else:
                    break
            if last_pair:
                # even head's AV contribution for this chunk
                kt0, ktn = TKC[tci]
                he = 2 * p
                for ti, (qt0, qtn) in enumerate(TQ):
                    nc.tensor.matmul(
                        pa_last[ti][:2 * DH, :qtn],
                        v_tiles[b][:ktn, tci, he, :],
                        pt_tiles[(b, he)][tci][:ktn, qt0:qt0 + qtn],
                        start=(tci == 0), stop=(tci == len(TKC) - 1),
                        skip_group_check=True,
                    )
        if last_pair:
            for ti in range(2):
                av_epilogue(pa_last[ti], b, 2 * p, ti)
            del pt_tiles[(b, 2 * p)]
            for ti in range(2):
                av_qt(b, 2 * p + 1, ti, pool=pconv)
            continue
        for h in (2 * p, 2 * p + 1):
            for ti in range(2):
                if fi >= v_done_idx[b]:
                    avq.append((pi, b, h, ti))
                else:
                    pending_av.append((pi, b, h, ti))
        promote_pending()

    # drain
    while fi < len(F):
        emit_f()
    while pending_av:
        avq.append(pending_av.popleft())
    while avq:
        _, bb, hh, ti = avq.popleft()
        av_qt(bb, hh, ti)
    ctx.close()


_CACHE = {}


def _build():
    key = "nc"
    if key in _CACHE:
        return _CACHE[key]
    nc = bacc.Bacc("TRN2", target_bir_lowering=False, debug=False,
                   num_devices=NCORES)
    xs = nc.dram_tensor("xs", [BLOC, D, L], MMDT, kind="ExternalInput").ap()
    wq = nc.dram_tensor("wqt", [128, NIC, NIC, KW, 128], MMDT, kind="ExternalInput").ap()
    wk = nc.dram_tensor("wkt", [128, NIC, NIC, KW, 128], MMDT, kind="ExternalInput").ap()
    wv = nc.dram_tensor("wvt", [128, NIC, KW, D], MMDT, kind="ExternalInput").ap()
    bq = nc.dram_tensor("bq", [128, NIC], F32, kind="ExternalInput").ap()
    bk = nc.dram_tensor("bk", [128, NIC], F32, kind="ExternalInput").ap()
    bv = nc.dram_tensor("bv", [128, D], F32, kind="ExternalInput").ap()
    out = nc.dram_tensor("out", [BLOC, D, LP], BF16, kind="ExternalOutput").ap()
    with tile.TileContext(nc) as tc:
        _emit(tc, xs, wq, wk, wv, bq, bk, bv, out)
    nc.compile()
    _CACHE[key] = nc
    return nc


def _wt(w):
    # w: [O, I, K] -> [p, ic, k, o] with i = ic*128 + p
    return np.ascontiguousarray(
        w.transpose(1, 2, 0).reshape(NIC, 128, KW, D).transpose(1, 0, 2, 3)
    ).astype(MMDT_NP)


def _wt_qk(w):
    # w: [O, I, K] -> oc-major [p, oc, ic, k, o'] with i = ic*128 + p,
    # o = oc*128 + o'  (3KB contiguous per partition per oc tile)
    arr = np.asarray(w, np.float32).transpose(1, 2, 0)          # [I, K, O]
    arr = arr.reshape(NIC, 128, KW, NIC, 128)                   # [ic,p,kk,oc,o']
    return np.ascontiguousarray(arr.transpose(1, 3, 0, 2, 4)).astype(MMDT_NP)


def _in_common(w0, b0, w1, b1, w2, b2):
    return {
        "wqt": _wt_qk(np.asarray(w0, np.float32)),
        "wkt": _wt_qk(np.asarray(w1, np.float32)),
        "wvt": _wt(np.asarray(w2, np.float32)),
        # per-partition bias layouts: [p, oc] with o = oc*128+p
        "bq": np.ascontiguousarray(
            np.asarray(b0, np.float32).reshape(NIC, 128).T),
        "bk": np.ascontiguousarray(
            np.asarray(b1, np.float32).reshape(NIC, 128).T),
        "bv": np.ascontiguousarray(
            np.tile(np.asarray(b2, np.float32)[None, :], (128, 1))),
    }


def kernel(x, w0, b0, w1, b1, w2, b2):
    x = np.asarray(x, dtype=np.float32)
    inp_common = _in_common(w0, b0, w1, b1, w2, b2)
    nc = _build()
    in_maps = [
        {"xs": np.ascontiguousarray(x[c * BLOC:(c + 1) * BLOC]).astype(MMDT_NP), **inp_common}
        for c in range(NCORES)
    ]
    res = run_bass_kernel_spmd(nc, in_maps, list(range(NCORES)))
    return np.concatenate(
        [res.results[c]["out"].astype(np.float32) for c in range(NCORES)], axis=0)


def run_traced(x, w0, b0, w1, b1, w2, b2, **kw):
    """Like kernel() but returns (output, BassKernelResults)."""
    x = np.asarray(x, dtype=np.float32)
    inp_common = _in_common(w0, b0, w1, b1, w2, b2)
    nc = _build()
    in_maps = [
        {"xs": np.ascontiguousarray(x[c * BLOC:(c + 1) * BLOC]).astype(MMDT_NP), **inp_common}
        for c in range(NCORES)
    ]
    res = run_bass_kernel_spmd(nc, in_maps, list(range(NCORES)), **kw)
    out = np.concatenate(
        [res.results[c]["out"].astype(np.float32) for c in range(NCORES)], axis=0)
    return out, res
